# revision 2
# baseline (speedup 1.0000x reference)
"""LMMD (DSAN local MMD) loss on 8 Trainium2 NeuronCores — v2.

Math:
    X = concat(source, target)                  # [N=4096, D=1024]
    l2[i,j] = sq_i + sq_j - 2 x_i.x_j
    bw      = sum(l2) / (N^2 - N) / 4
    K       = sum_q exp(-l2 / (bw 2^q)), q=0..4
    loss    = sum_c v_c^T K v_c / 12,  V = [s_norm; -t_norm]

Decomposition: the 32x32 grid of 128-row tiles has 528 unordered
tile-pairs (incl. diagonal).  Tiles form 8 groups of 4 (group g = tiles
4g..4g+3).  Core c covers, with the i-side always in its own group:
    -  within-group pairs of group c                    (10 units)
    -  all pairs vs groups c+1, c+2, c+3                (3 x 16)
    -  half the pairs vs group c+4 (2x2-block split)    (8)
    => 66 m-units (128x128 tile-pairs) per core, exact cover of 528.
Each core touches only 20 of 32 tiles => ~24KB/partition total DMA.

Device program (uniform across cores; per-core data relabeled on host):
    20 j-strips, m-seq [4,3,2,1 | 2,2,2 | 4 x12 | 2]; strips pack into 6
    PSUM G-slabs (<=3 banks).  Per strip: 4 fp8 DoubleRow matmuls
    (contraction 1024 = 4 pairs of 128-chunks) -> G[j,i] slab slice.
    Per slab: ACT u = exp(2 c4 G) fp16; squaring ladder u2,u4,u8,u16
    with columns split DVE/Pool; per (strip, own-tile t, q): PE matmul,
    stationary lhsT = u^(2^(4-q)) 128x128 slice, moving rhs =
    V'_q[j-tile] [128,12] fp16 -> R[i,(t,q)] += in PSUM ([128,240], one
    bank).  V'_q[j] = w2 * V[j] * exp(-c_q sq_j) host-precomputed
    (w2 = 2, or 1 on the diagonal tile).
Host: loss = 1/12 sum_q sum_i exp(-c_q sq_i) V[i,:] . R_q[i,:].
bw analytically: sum(l2) = 2N sum(sq) - 2|colsum|^2 (the relu clamp
only affects the exactly-zero diagonal).
"""

import numpy as np
import ml_dtypes

import concourse.bass as bass
from concourse import bacc
import concourse.mybir as mybir
import concourse.tile as tile
from concourse.bass_utils import run_bass_kernel_spmd

B = 2048
D = 1024
C = 12
NCORES = 8
N = 2 * B
NQ = 5
NT = 32                  # 128-row tiles in N
NSLOT = 20               # j-strips per core
NW8 = 16                 # non-own strips with DMA'd weights
KS = 8                   # 128-row contraction chunks
KP = 4                   # DoubleRow chunk-pairs

# strip slot -> (i_lo, i_hi) in units of 128 own-i columns
# slots: 0-3 own-group (alpha), 4-5 H0,H1 (lo), 6-17 B0..B11,
# 18-19 H2,H3 (hi; last so the t0/t1 R-chains close earlier)
TR = {0: (0, 4), 1: (1, 4), 2: (2, 4), 3: (3, 4),
      4: (0, 2), 5: (0, 2), 18: (2, 4), 19: (2, 4)}
for _s in range(6, 18):
    TR[_s] = (0, 4)
# diagonal 128x128 block (slot, t): weight 1x instead of 2x
DIAG = {(0, 0), (1, 1), (2, 2), (3, 3)}
# slabs: ordered slot lists (unit widths 10,8,12,12,12,12; 66 total).
# S0 is pure own-group so its gram needs only the first (4KB) DMA.
SLABS = [[0, 1, 3, 2], [4, 5, 6], [7, 8, 9],
         [10, 11, 12], [13, 14, 15], [16, 17, 18, 19]]
LAST_FOR_T = {0: 17, 1: 17, 2: 19, 3: 19}
# ladder columns handed to GpSimd per (slab, level); multiple of 128 so
# the d/p chains live in disjoint tiles and never cross-synchronize.
# Constant within a slab so each piece reads one parent piece.
def _pool_cols(si, li, W):
    return 128


ACT_UNITS = {(1, 4), (2, 4), (3, 4), (4, 4)}   # (slab, level) on ACT Square
NWARM = 16               # dummy PE matmuls to ramp the PE clock during DMA
WDELAY = 3               # slabs of delay before emitting weighted matmuls

_BUILT = None


def _core_jlist(c):
    """Physical j-tile index per strip slot for core c."""
    p = (c + 4) % 8
    own = [4 * c + 0, 4 * c + 1, 4 * c + 2, 4 * c + 3]
    if c < 4:
        h = [4 * p + 0, 4 * p + 1, 4 * p + 2, 4 * p + 3]   # H0..H3
    else:
        h = [4 * p + 2, 4 * p + 3, 4 * p + 0, 4 * p + 1]
    bs = []
    for dg in (1, 2, 3):
        g = (c + dg) % 8
        bs += [4 * g + 0, 4 * g + 1, 4 * g + 2, 4 * g + 3]
    return own + [h[0], h[1]] + bs + [h[2], h[3]]


def _build_program(skip_weighted=False, skip_ladder=False, skip_gram=False,
                   skip_dma=False):
    fp32 = mybir.dt.float32
    fp16 = mybir.dt.float16
    fp8 = mybir.dt.float8e4
    Exp = mybir.ActivationFunctionType.Exp
    DR = mybir.MatmulPerfMode.DoubleRow

    nc = bacc.Bacc()
    # w8[p, ws, ks, jj] = Xq[J(ws+4)*128+jj, ks*128+p]   (slots 4..19)
    w8 = nc.declare_dram_parameter("w8", [128, NW8, KS, 128], fp8, isOutput=False)
    # own8[p, ks, t*128+ii] = Xq[(4c+t)*128+ii, ks*128+p]
    own8 = nc.declare_dram_parameter("own8", [128, KS, 512], fp8, isOutput=False)
    # vt2[p, (q*NSLOT+slot)*C+cls] ; vt1[p, (q*4+slot)*C+cls]
    vt2 = nc.declare_dram_parameter("vt2", [128, NQ * NSLOT * C], fp16, isOutput=False)
    vt1 = nc.declare_dram_parameter("vt1", [128, NQ * 4 * C], fp16, isOutput=False)
    scl = nc.declare_dram_parameter("scl", [128, 8], fp32, isOutput=False)
    rout = nc.declare_dram_parameter("r_out", [128, 4 * NQ * C], fp32, isOutput=True)

    slab_off = {}
    slab_w = []
    for slots in SLABS:
        off = 0
        for s in slots:
            slab_off[s] = off
            off += TR[s][1] - TR[s][0]
        slab_w.append(off * 128)

    with tile.TileContext(nc) as tc:
        with (
            tc.tile_pool(name="singles", bufs=1) as singles,
            tc.tile_pool(name="fpool", bufs=6) as fpool,
            tc.tile_pool(name="gpsum", bufs=2, space="PSUM") as gpsum,
            tc.tile_pool(name="rpsum", bufs=1, space="PSUM") as rpsum,
        ):
            own_sb = singles.tile([128, KS, 512], fp8)
            nc.sync.dma_start(out=own_sb[:, 0:4], in_=own8[:, 0:4])
            nc.sync.dma_start(out=own_sb[:, 4:8], in_=own8[:, 4:8])
            scl_sb = singles.tile([128, 8], fp32)
            nc.sync.dma_start(out=scl_sb[:], in_=scl[:])
            # Warm ACT so the Exp table load happens during the DMA head,
            # not in front of the first real exp.
            warm = singles.tile([128, 8], fp32)
            nc.scalar.activation(warm[:], scl_sb[:], Exp)
            w8_sb = singles.tile([128, NW8 * KS, 128], fp8)
            if not skip_dma:
                for lo, hi in ((0, 3), (3, 6), (6, 9), (9, 12), (12, 16)):
                    nc.sync.dma_start(out=w8_sb[:, lo * KS:hi * KS],
                                      in_=w8[:, lo:hi])
            else:
                nc.vector.memset(w8_sb[:], 0)
            # vt tiles are only read by the weighted matmuls, which trail the
            # ladder; load them after the gram weights.
            vt1_sb = singles.tile([128, NQ * 4 * C], fp16)
            nc.sync.dma_start(out=vt1_sb[:], in_=vt1[:])
            vt2_sb = singles.tile([128, NQ * NSLOT * C], fp16)
            nc.sync.dma_start(out=vt2_sb[:], in_=vt2[:])

            r_ps = rpsum.tile([128, 4 * NQ * C], fp32, name="racc")
            # The real walrus lowering mishandles interleaved accumulation
            # groups on one PSUM tile; zero once and accumulate without
            # per-chain start flags instead.
            nc.vector.memset(r_ps[:], 0)

            # PE clock warmup: dummy matmuls with no DMA dependency keep the
            # PE continuously busy through the DMA head so the real gram runs
            # at full clock (the cost model ramps 0.65->2.4GHz over 3us).
            dumw = singles.tile([128, 2, 256], fp8)
            nc.gpsimd.memset(dumw[:], 0)
            dps = rpsum.tile([128, 256], fp32, name="dwarm")
            for i in range(NWARM):
                nc.tensor.matmul(dps[:], lhsT=dumw[:, :, 0:128], rhs=dumw[:],
                                 start=True, stop=True, perf_mode=DR)

            def lhsT_for(s, kp):
                if s < 4:
                    return own_sb[:, 2 * kp:2 * kp + 2, s * 128:(s + 1) * 128]
                base = (s - 4) * KS + 2 * kp
                return w8_sb[:, base:base + 2, :]

            levels = []          # per slab: [u1, u2, u4, u8, u16]

            def emit_weighted(si):
                if skip_weighted:
                    return
                for s in SLABS[si]:
                    lo, hi = TR[s]
                    goff = slab_off[s] * 128
                    for q in range(NQ):
                        pieces = levels[si][4 - q]
                        for t in range(lo, hi):
                            col = goff + (t - lo) * 128
                            for tile_, plo, phi in pieces:
                                if plo <= col and col + 128 <= phi:
                                    if len(pieces) == 1:
                                        f = tile_[:, col:col + 128]
                                    else:
                                        f = tile_[:, col - plo:col - plo + 128]
                                    break
                            else:
                                raise AssertionError("slice straddles pieces")
                            if (s, t) in DIAG:
                                v = vt1_sb[:, (q * 4 + s) * C:(q * 4 + s + 1) * C]
                            else:
                                v = vt2_sb[:, (q * NSLOT + s) * C:(q * NSLOT + s + 1) * C]
                            nc.tensor.matmul(
                                r_ps[:, (t * NQ + q) * C:(t * NQ + q + 1) * C],
                                lhsT=f,
                                rhs=v,
                                start=False,
                                stop=(s == LAST_FOR_T[t]),
                                skip_group_check=True,
                            )

            for si, slots in enumerate(SLABS):
                W = slab_w[si]
                g = gpsum.tile([128, 1536], fp32)
                if not skip_gram:
                    for s in slots:
                        lo, hi = TR[s]
                        m = hi - lo
                        goff = slab_off[s] * 128
                        for kp in range(KP):
                            nc.tensor.matmul(
                                g[:, goff:goff + m * 128],
                                lhsT=lhsT_for(s, kp),
                                rhs=own_sb[:, 2 * kp:2 * kp + 2, lo * 128:hi * 128],
                                start=(kp == 0),
                                stop=(kp == KP - 1),
                                perf_mode=DR,
                            )
                else:
                    for wo in range(0, W, 512):
                        nc.tensor.matmul(
                            g[:, wo:wo + 512],
                            lhsT=own_sb[:, 0:2, 0:128],
                            rhs=own_sb[:, 0:2, 0:512],
                            start=True, stop=True, perf_mode=DR)
                u1 = fpool.tile([128, 1536], fp16, tag="u1", bufs=6)
                nc.scalar.activation(u1[:, 0:W], g[:, 0:W], Exp,
                                     scale=scl_sb[:, 0:1])
                # levels are stored as piece lists [(tile, lo, hi), ...] in
                # column order; d/p pieces are disjoint tiles so the DVE and
                # Pool chains never wait on each other
                lv = [[(u1, 0, W)]]
                for li, nm in enumerate(("u2", "u4", "u8", "u16"), start=1):
                    d = W - _pool_cols(si, li, W)
                    td = fpool.tile([128, 1536], fp16, tag=nm + "d", bufs=6)
                    tp = fpool.tile([128, 384], fp16, tag=nm + "p", bufs=6)
                    prev = lv[-1]

                    def piece_ap(col0, col1):
                        # view of [col0:col1) in the previous level; may span
                        # the previous split as long as it stays in one piece
                        for tile_, lo, hi in prev:
                            if lo <= col0 and col1 <= hi:
                                if len(prev) == 1:
                                    return tile_[:, col0:col1]
                                return tile_[:, col0 - lo:col1 - lo]
                        return None

                    if not skip_ladder:
                        pd = piece_ap(0, d)
                        pp = piece_ap(d, W)
                        if pd is None or pp is None:
                            raise AssertionError("piece straddles split")
                        if (si, li) in ACT_UNITS:
                            nc.scalar.square(td[:, 0:d], pd)
                            nc.scalar.square(tp[:, 0:W - d], pp)
                        else:
                            nc.vector.tensor_mul(td[:, 0:d], pd, pd)
                            nc.gpsimd.tensor_mul(tp[:, 0:W - d], pp, pp)
                    lv.append([(td, 0, d), (tp, d, W)])
                levels.append(lv)
                if si >= WDELAY:
                    emit_weighted(si - WDELAY)
            for si in range(max(0, len(SLABS) - WDELAY), len(SLABS)):
                emit_weighted(si)

            if skip_weighted:
                nc.tensor.matmul(r_ps[:, 0:C],
                                 lhsT=levels[0][0][0][0][:, 0:128],
                                 rhs=vt1_sb[:, 0:C], start=True, stop=True)

            # Stage + store R in two halves: the t0/t1 chains close at slot
            # 17 (last B), t2/t3 only after the trailing H strips.
            stage = singles.tile([128, 4 * NQ * C], fp32)
            half = 2 * NQ * C
            nc.vector.tensor_copy(stage[:], r_ps[:])
            nc.sync.dma_start(out=rout[:], in_=stage[:])

    nc.compile()
    return nc


def _prep(source, target, source_label, target_logits):
    X = np.concatenate([np.asarray(source), np.asarray(target)], axis=0)
    X64 = X.astype(np.float64)
    sq = np.einsum("nd,nd->n", X64, X64)
    colsum = X64.sum(axis=0)
    sum_l2 = 2.0 * N * sq.sum() - 2.0 * (colsum @ colsum)
    bw = sum_l2 / (N * N - N) / 4.0
    cq = np.array([1.0 / (bw * 2.0**q) for q in range(NQ)])  # [5]

    sl = np.asarray(source_label, np.float64)
    tl = np.asarray(target_logits, np.float64)
    ssum = sl.sum(0)
    s_norm = np.where(ssum > 0, sl / np.where(ssum > 0, ssum, 1.0), 0.0)
    tsum = tl.sum(0)
    t_norm = np.where(tsum > 0, tl / np.where(tsum > 0, tsum, 1.0), 0.0)
    s_pres = np.zeros(C)
    np.add.at(s_pres, sl.argmax(1), 1.0)
    t_pres = np.zeros(C)
    np.add.at(t_pres, tl.argmax(1), 1.0)
    common = ((s_pres > 0) & (t_pres > 0)).astype(np.float64)
    V = np.concatenate([s_norm * common, -t_norm * common], axis=0)  # [N, C]

    Xq = X.astype(ml_dtypes.float8_e4m3)
    # global weight layout [128(p), 32(jt), 8(ks), 128(jj)]
    w8g = np.ascontiguousarray(
        Xq.T.reshape(KS, 128, NT, 128).transpose(1, 2, 0, 3))
    beta = np.exp(-np.outer(cq, sq))                        # [5, N]
    return X, sq, cq, V, Xq, w8g, beta


def _core_inputs(c, w8g, V, beta, cq):
    jl = _core_jlist(c)
    w8 = np.ascontiguousarray(w8g[:, jl[4:]])               # [128, 16, 8, 128]
    own8 = np.ascontiguousarray(
        w8g[:, 4 * c:4 * c + 4].transpose(0, 2, 1, 3).reshape(128, KS, 512))
    vt2 = np.empty((128, NQ * NSLOT * C), np.float16)
    vt1 = np.empty((128, NQ * 4 * C), np.float16)
    for q in range(NQ):
        for s, J in enumerate(jl):
            blk = V[J * 128:(J + 1) * 128] * beta[q, J * 128:(J + 1) * 128][:, None]
            vt2[:, (q * NSLOT + s) * C:(q * NSLOT + s + 1) * C] = 2.0 * blk
            if s < 4:
                vt1[:, (q * 4 + s) * C:(q * 4 + s + 1) * C] = blk
    scl = np.zeros((128, 8), np.float32)
    scl[:, 0] = 2.0 * cq[4]
    return {"w8": w8, "own8": own8, "vt2": vt2, "vt1": vt1, "scl": scl}


def _postprocess(results, sq, cq, V, beta):
    # r_out[p, (t*5+q)*C+cls] = sum_j V'_q[j,cls] u^(2^(4-q))[j, i=(4c+t)*128+p]
    loss = 0.0
    for c in range(NCORES):
        r = np.asarray(results[c]["r_out"], np.float64).reshape(128, 4, NQ, C)
        for t in range(4):
            i0 = (4 * c + t) * 128
            Vi = V[i0:i0 + 128]                    # [128, C]
            a = beta[:, i0:i0 + 128]               # [5, 128] alpha_q(i)
            loss += np.einsum("qp,pc,pqc->", a, Vi, r[:, t])
    return loss / C


def _run(in_maps, trace=False, **kw):
    global _BUILT
    if _BUILT is None:
        _BUILT = _build_program()
    return run_bass_kernel_spmd(_BUILT, in_maps, list(range(NCORES)), trace=trace, **kw)


def kernel(source, target, source_label, target_logits, _trace=False, _ret_bkr=False):
    X, sq, cq, V, Xq, w8g, beta = _prep(source, target, source_label, target_logits)
    in_maps = [_core_inputs(c, w8g, V, beta, cq) for c in range(NCORES)]
    bkr = _run(in_maps, trace=_trace)
    loss = _postprocess(bkr.results, sq, cq, V, beta)
    out = np.float32(loss)
    if _ret_bkr:
        return out, bkr
    return out


# revision 3
# speedup vs baseline: 1.0085x; 1.0085x over previous
"""LMMD (DSAN local MMD) loss on 8 Trainium2 NeuronCores — v2.

Math:
    X = concat(source, target)                  # [N=4096, D=1024]
    l2[i,j] = sq_i + sq_j - 2 x_i.x_j
    bw      = sum(l2) / (N^2 - N) / 4
    K       = sum_q exp(-l2 / (bw 2^q)), q=0..4
    loss    = sum_c v_c^T K v_c / 12,  V = [s_norm; -t_norm]

Decomposition: the 32x32 grid of 128-row tiles has 528 unordered
tile-pairs (incl. diagonal).  Tiles form 8 groups of 4 (group g = tiles
4g..4g+3).  Core c covers, with the i-side always in its own group:
    -  within-group pairs of group c                    (10 units)
    -  all pairs vs groups c+1, c+2, c+3                (3 x 16)
    -  half the pairs vs group c+4 (2x2-block split)    (8)
    => 66 m-units (128x128 tile-pairs) per core, exact cover of 528.
Each core touches only 20 of 32 tiles => ~24KB/partition total DMA.

Device program (uniform across cores; per-core data relabeled on host):
    20 j-strips, m-seq [4,3,2,1 | 2,2,2 | 4 x12 | 2]; strips pack into 6
    PSUM G-slabs (<=3 banks).  Per strip: 4 fp8 DoubleRow matmuls
    (contraction 1024 = 4 pairs of 128-chunks) -> G[j,i] slab slice.
    Per slab: ACT u = exp(2 c4 G) fp16; squaring ladder u2,u4,u8,u16
    with columns split DVE/Pool; per (strip, own-tile t, q): PE matmul,
    stationary lhsT = u^(2^(4-q)) 128x128 slice, moving rhs =
    V'_q[j-tile] [128,12] fp16 -> R[i,(t,q)] += in PSUM ([128,240], one
    bank).  V'_q[j] = w2 * V[j] * exp(-c_q sq_j) host-precomputed
    (w2 = 2, or 1 on the diagonal tile).
Host: loss = 1/12 sum_q sum_i exp(-c_q sq_i) V[i,:] . R_q[i,:].
bw analytically: sum(l2) = 2N sum(sq) - 2|colsum|^2 (the relu clamp
only affects the exactly-zero diagonal).
"""

import numpy as np
import ml_dtypes

import concourse.bass as bass
from concourse import bacc
import concourse.mybir as mybir
import concourse.tile as tile
from concourse.bass_utils import run_bass_kernel_spmd

B = 2048
D = 1024
C = 12
NCORES = 8
N = 2 * B
NQ = 5
NT = 32                  # 128-row tiles in N
NSLOT = 20               # j-strips per core
NW8 = 16                 # non-own strips with DMA'd weights
KS = 8                   # 128-row contraction chunks
KP = 4                   # DoubleRow chunk-pairs

# strip slot -> (i_lo, i_hi) in units of 128 own-i columns
# slots: 0-3 own-group (alpha), 4-5 H0,H1 (lo), 6-17 B0..B11,
# 18-19 H2,H3 (hi; last so the t0/t1 R-chains close earlier)
TR = {0: (0, 4), 1: (1, 4), 2: (2, 4), 3: (3, 4),
      4: (0, 2), 5: (0, 2), 18: (2, 4), 19: (2, 4)}
for _s in range(6, 18):
    TR[_s] = (0, 4)
# diagonal 128x128 block (slot, t): weight 1x instead of 2x
DIAG = {(0, 0), (1, 1), (2, 2), (3, 3)}
# slabs: ordered slot lists (unit widths 10,8,12,12,12,12; 66 total).
# S0 is pure own-group so its gram needs only the first (4KB) DMA.
SLABS = [[0, 1, 3, 2], [4, 5, 6], [7, 8, 9],
         [10, 11, 12], [13, 14, 15], [16, 17, 18, 19]]
LAST_FOR_T = {0: 17, 1: 17, 2: 19, 3: 19}
# ladder columns handed to GpSimd per (slab, level); multiple of 128 so
# the d/p chains live in disjoint tiles and never cross-synchronize.
# Constant within a slab so each piece reads one parent piece.
def _pool_cols(si, li, W):
    return 256 if si <= 3 else 128


ACT_UNITS = {(1, 4), (2, 4), (3, 4), (4, 4)}   # (slab, level) on ACT Square
NWARM = 16               # dummy PE matmuls to ramp the PE clock during DMA
WDELAY = 3               # slabs of delay before emitting weighted matmuls

_BUILT = None


def _core_jlist(c):
    """Physical j-tile index per strip slot for core c."""
    p = (c + 4) % 8
    own = [4 * c + 0, 4 * c + 1, 4 * c + 2, 4 * c + 3]
    if c < 4:
        h = [4 * p + 0, 4 * p + 1, 4 * p + 2, 4 * p + 3]   # H0..H3
    else:
        h = [4 * p + 2, 4 * p + 3, 4 * p + 0, 4 * p + 1]
    bs = []
    for dg in (1, 2, 3):
        g = (c + dg) % 8
        bs += [4 * g + 0, 4 * g + 1, 4 * g + 2, 4 * g + 3]
    return own + [h[0], h[1]] + bs + [h[2], h[3]]


def _build_program(skip_weighted=False, skip_ladder=False, skip_gram=False,
                   skip_dma=False):
    fp32 = mybir.dt.float32
    fp16 = mybir.dt.float16
    fp8 = mybir.dt.float8e4
    Exp = mybir.ActivationFunctionType.Exp
    DR = mybir.MatmulPerfMode.DoubleRow

    nc = bacc.Bacc()
    # w8[p, ws, ks, jj] = Xq[J(ws+4)*128+jj, ks*128+p]   (slots 4..19)
    w8 = nc.declare_dram_parameter("w8", [128, NW8, KS, 128], fp8, isOutput=False)
    # own8[p, ks, t*128+ii] = Xq[(4c+t)*128+ii, ks*128+p]
    own8 = nc.declare_dram_parameter("own8", [128, KS, 512], fp8, isOutput=False)
    # vt2[p, (q*NSLOT+slot)*C+cls] ; vt1[p, (q*4+slot)*C+cls]
    vt2 = nc.declare_dram_parameter("vt2", [128, NQ * NSLOT * C], fp16, isOutput=False)
    vt1 = nc.declare_dram_parameter("vt1", [128, NQ * 4 * C], fp16, isOutput=False)
    scl = nc.declare_dram_parameter("scl", [128, 8], fp32, isOutput=False)
    rout = nc.declare_dram_parameter("r_out", [128, 4 * NQ * C], fp32, isOutput=True)

    slab_off = {}
    slab_w = []
    for slots in SLABS:
        off = 0
        for s in slots:
            slab_off[s] = off
            off += TR[s][1] - TR[s][0]
        slab_w.append(off * 128)

    with tile.TileContext(nc) as tc:
        with (
            tc.tile_pool(name="singles", bufs=1) as singles,
            tc.tile_pool(name="fpool", bufs=6) as fpool,
            tc.tile_pool(name="gpsum", bufs=2, space="PSUM") as gpsum,
            tc.tile_pool(name="rpsum", bufs=1, space="PSUM") as rpsum,
        ):
            own_sb = singles.tile([128, KS, 512], fp8)
            nc.sync.dma_start(out=own_sb[:, 0:4], in_=own8[:, 0:4])
            nc.sync.dma_start(out=own_sb[:, 4:8], in_=own8[:, 4:8])
            scl_sb = singles.tile([128, 8], fp32)
            nc.sync.dma_start(out=scl_sb[:], in_=scl[:])
            # Warm ACT so the Exp table load happens during the DMA head,
            # not in front of the first real exp.
            warm = singles.tile([128, 8], fp32)
            nc.scalar.activation(warm[:], scl_sb[:], Exp)
            w8_sb = singles.tile([128, NW8 * KS, 128], fp8)
            if not skip_dma:
                for lo, hi in ((0, 3), (3, 6), (6, 9), (9, 12), (12, 16)):
                    nc.sync.dma_start(out=w8_sb[:, lo * KS:hi * KS],
                                      in_=w8[:, lo:hi])
            else:
                nc.vector.memset(w8_sb[:], 0)
            # vt tiles are only read by the weighted matmuls, which trail the
            # ladder; load them after the gram weights.
            vt1_sb = singles.tile([128, NQ * 4 * C], fp16)
            nc.sync.dma_start(out=vt1_sb[:], in_=vt1[:])
            vt2_sb = singles.tile([128, NQ * NSLOT * C], fp16)
            nc.sync.dma_start(out=vt2_sb[:], in_=vt2[:])

            r_ps = rpsum.tile([128, 4 * NQ * C], fp32, name="racc")
            # The real walrus lowering mishandles interleaved accumulation
            # groups on one PSUM tile; zero once and accumulate without
            # per-chain start flags instead.
            nc.vector.memset(r_ps[:], 0)

            # PE clock warmup: dummy matmuls with no DMA dependency keep the
            # PE continuously busy through the DMA head so the real gram runs
            # at full clock (the cost model ramps 0.65->2.4GHz over 3us).
            dumw = singles.tile([128, 2, 256], fp8)
            nc.gpsimd.memset(dumw[:], 0)
            dps = rpsum.tile([128, 256], fp32, name="dwarm")
            for i in range(NWARM):
                nc.tensor.matmul(dps[:], lhsT=dumw[:, :, 0:128], rhs=dumw[:],
                                 start=True, stop=True, perf_mode=DR)

            def lhsT_for(s, kp):
                if s < 4:
                    return own_sb[:, 2 * kp:2 * kp + 2, s * 128:(s + 1) * 128]
                base = (s - 4) * KS + 2 * kp
                return w8_sb[:, base:base + 2, :]

            levels = []          # per slab: [u1, u2, u4, u8, u16]

            def emit_weighted(si):
                if skip_weighted:
                    return
                for s in SLABS[si]:
                    lo, hi = TR[s]
                    goff = slab_off[s] * 128
                    for q in range(NQ):
                        pieces = levels[si][4 - q]
                        for t in range(lo, hi):
                            col = goff + (t - lo) * 128
                            for tile_, plo, phi in pieces:
                                if plo <= col and col + 128 <= phi:
                                    if len(pieces) == 1:
                                        f = tile_[:, col:col + 128]
                                    else:
                                        f = tile_[:, col - plo:col - plo + 128]
                                    break
                            else:
                                raise AssertionError("slice straddles pieces")
                            if (s, t) in DIAG:
                                v = vt1_sb[:, (q * 4 + s) * C:(q * 4 + s + 1) * C]
                            else:
                                v = vt2_sb[:, (q * NSLOT + s) * C:(q * NSLOT + s + 1) * C]
                            nc.tensor.matmul(
                                r_ps[:, (t * NQ + q) * C:(t * NQ + q + 1) * C],
                                lhsT=f,
                                rhs=v,
                                start=False,
                                stop=(s == LAST_FOR_T[t]),
                                skip_group_check=True,
                            )

            for si, slots in enumerate(SLABS):
                W = slab_w[si]
                g = gpsum.tile([128, 1536], fp32)
                if not skip_gram:
                    for s in slots:
                        lo, hi = TR[s]
                        m = hi - lo
                        goff = slab_off[s] * 128
                        for kp in range(KP):
                            nc.tensor.matmul(
                                g[:, goff:goff + m * 128],
                                lhsT=lhsT_for(s, kp),
                                rhs=own_sb[:, 2 * kp:2 * kp + 2, lo * 128:hi * 128],
                                start=(kp == 0),
                                stop=(kp == KP - 1),
                                perf_mode=DR,
                            )
                else:
                    for wo in range(0, W, 512):
                        nc.tensor.matmul(
                            g[:, wo:wo + 512],
                            lhsT=own_sb[:, 0:2, 0:128],
                            rhs=own_sb[:, 0:2, 0:512],
                            start=True, stop=True, perf_mode=DR)
                u1 = fpool.tile([128, 1536], fp16, tag="u1", bufs=6)
                nc.scalar.activation(u1[:, 0:W], g[:, 0:W], Exp,
                                     scale=scl_sb[:, 0:1])
                # levels are stored as piece lists [(tile, lo, hi), ...] in
                # column order; d/p pieces are disjoint tiles so the DVE and
                # Pool chains never wait on each other
                lv = [[(u1, 0, W)]]
                for li, nm in enumerate(("u2", "u4", "u8", "u16"), start=1):
                    d = W - _pool_cols(si, li, W)
                    td = fpool.tile([128, 1536], fp16, tag=nm + "d", bufs=6)
                    tp = fpool.tile([128, 384], fp16, tag=nm + "p", bufs=6)
                    prev = lv[-1]

                    def piece_ap(col0, col1):
                        # view of [col0:col1) in the previous level; may span
                        # the previous split as long as it stays in one piece
                        for tile_, lo, hi in prev:
                            if lo <= col0 and col1 <= hi:
                                if len(prev) == 1:
                                    return tile_[:, col0:col1]
                                return tile_[:, col0 - lo:col1 - lo]
                        return None

                    if not skip_ladder:
                        pd = piece_ap(0, d)
                        pp = piece_ap(d, W)
                        if pd is None or pp is None:
                            raise AssertionError("piece straddles split")
                        if (si, li) in ACT_UNITS:
                            nc.scalar.square(td[:, 0:d], pd)
                            nc.gpsimd.tensor_mul(tp[:, 0:W - d], pp, pp)
                        else:
                            nc.vector.tensor_mul(td[:, 0:d], pd, pd)
                            nc.gpsimd.tensor_mul(tp[:, 0:W - d], pp, pp)
                    lv.append([(td, 0, d), (tp, d, W)])
                levels.append(lv)
                if si >= WDELAY:
                    emit_weighted(si - WDELAY)
            for si in range(max(0, len(SLABS) - WDELAY), len(SLABS)):
                emit_weighted(si)

            if skip_weighted:
                nc.tensor.matmul(r_ps[:, 0:C],
                                 lhsT=levels[0][0][0][0][:, 0:128],
                                 rhs=vt1_sb[:, 0:C], start=True, stop=True)

            # Stage + store R in two halves: the t0/t1 chains close at slot
            # 17 (last B), t2/t3 only after the trailing H strips.
            stage = singles.tile([128, 4 * NQ * C], fp32)
            half = 2 * NQ * C
            nc.vector.tensor_copy(stage[:], r_ps[:])
            nc.sync.dma_start(out=rout[:], in_=stage[:])

    nc.compile()
    return nc


def _prep(source, target, source_label, target_logits):
    X = np.concatenate([np.asarray(source), np.asarray(target)], axis=0)
    X64 = X.astype(np.float64)
    sq = np.einsum("nd,nd->n", X64, X64)
    colsum = X64.sum(axis=0)
    sum_l2 = 2.0 * N * sq.sum() - 2.0 * (colsum @ colsum)
    bw = sum_l2 / (N * N - N) / 4.0
    cq = np.array([1.0 / (bw * 2.0**q) for q in range(NQ)])  # [5]

    sl = np.asarray(source_label, np.float64)
    tl = np.asarray(target_logits, np.float64)
    ssum = sl.sum(0)
    s_norm = np.where(ssum > 0, sl / np.where(ssum > 0, ssum, 1.0), 0.0)
    tsum = tl.sum(0)
    t_norm = np.where(tsum > 0, tl / np.where(tsum > 0, tsum, 1.0), 0.0)
    s_pres = np.zeros(C)
    np.add.at(s_pres, sl.argmax(1), 1.0)
    t_pres = np.zeros(C)
    np.add.at(t_pres, tl.argmax(1), 1.0)
    common = ((s_pres > 0) & (t_pres > 0)).astype(np.float64)
    V = np.concatenate([s_norm * common, -t_norm * common], axis=0)  # [N, C]

    Xq = X.astype(ml_dtypes.float8_e4m3)
    # global weight layout [128(p), 32(jt), 8(ks), 128(jj)]
    w8g = np.ascontiguousarray(
        Xq.T.reshape(KS, 128, NT, 128).transpose(1, 2, 0, 3))
    beta = np.exp(-np.outer(cq, sq))                        # [5, N]
    return X, sq, cq, V, Xq, w8g, beta


def _core_inputs(c, w8g, V, beta, cq):
    jl = _core_jlist(c)
    w8 = np.ascontiguousarray(w8g[:, jl[4:]])               # [128, 16, 8, 128]
    own8 = np.ascontiguousarray(
        w8g[:, 4 * c:4 * c + 4].transpose(0, 2, 1, 3).reshape(128, KS, 512))
    vt2 = np.empty((128, NQ * NSLOT * C), np.float16)
    vt1 = np.empty((128, NQ * 4 * C), np.float16)
    for q in range(NQ):
        for s, J in enumerate(jl):
            blk = V[J * 128:(J + 1) * 128] * beta[q, J * 128:(J + 1) * 128][:, None]
            vt2[:, (q * NSLOT + s) * C:(q * NSLOT + s + 1) * C] = 2.0 * blk
            if s < 4:
                vt1[:, (q * 4 + s) * C:(q * 4 + s + 1) * C] = blk
    scl = np.zeros((128, 8), np.float32)
    scl[:, 0] = 2.0 * cq[4]
    return {"w8": w8, "own8": own8, "vt2": vt2, "vt1": vt1, "scl": scl}


def _postprocess(results, sq, cq, V, beta):
    # r_out[p, (t*5+q)*C+cls] = sum_j V'_q[j,cls] u^(2^(4-q))[j, i=(4c+t)*128+p]
    loss = 0.0
    for c in range(NCORES):
        r = np.asarray(results[c]["r_out"], np.float64).reshape(128, 4, NQ, C)
        for t in range(4):
            i0 = (4 * c + t) * 128
            Vi = V[i0:i0 + 128]                    # [128, C]
            a = beta[:, i0:i0 + 128]               # [5, 128] alpha_q(i)
            loss += np.einsum("qp,pc,pqc->", a, Vi, r[:, t])
    return loss / C


def _run(in_maps, trace=False, **kw):
    global _BUILT
    if _BUILT is None:
        _BUILT = _build_program()
    return run_bass_kernel_spmd(_BUILT, in_maps, list(range(NCORES)), trace=trace, **kw)


def kernel(source, target, source_label, target_logits, _trace=False, _ret_bkr=False):
    X, sq, cq, V, Xq, w8g, beta = _prep(source, target, source_label, target_logits)
    in_maps = [_core_inputs(c, w8g, V, beta, cq) for c in range(NCORES)]
    bkr = _run(in_maps, trace=_trace)
    loss = _postprocess(bkr.results, sq, cq, V, beta)
    out = np.float32(loss)
    if _ret_bkr:
        return out, bkr
    return out
